# revision 9
# baseline (speedup 1.0000x reference)
"""Block2D attention on 8 TRN2 NeuronCores (fp16 compute, fp32 accum).

Sharding: data-parallel over the 8 independent (b, bnx, bny) attention blocks
(B=2 x bnx=2 x bny=2), one block of T=1024 tokens per core. No collectives.

Per-core pipeline: all main-loop matmuls run in (64,128) row-tiled mode so the
PE never pays a tile-mode-switch drain, and every slot is a 2-MM pair whose
moving operands are complementary partition halves of ONE tensor at the SAME
free offset -- the HW merges them into a single 512-col stream and runs both
tiles concurrently (measured ~216ns per pair = 2x):

  - projections (q/k/v): K=128 contraction split into k-lo (rows 0-63, psum
    bank P) and k-hi (rows 64-127, bank Q); out = P + Q via 2-step DVE evict.
  - QK^T: K=64 per head; head A on rows 0-63 -> bank qkA, head B on rows
    64-127 -> bank qkB; moving = qpair[:, n] union stream. This is the real
    2x win: one 512-col stream produces 256 score rows (two heads' scores).
  - exp on ACT as [128,1024] instructions (~1147ns each; ACT paces the main
    loop at ~293us total).
  - PV: token-chunk contraction split into t-lo/t-hi row halves -> banks X,Y;
    stationary vplus [tokens, 65] (64 v dims + ones column -> row 64 is the
    softmax denominator). out = X + Y via 2-step DVE evict into pair-packed
    ou [128, 1024] + den row kept at partition 64 (no partition shifts).
  - reciprocal via DRAM spread-bounce (all-lane DVE), GPSIMD normalize-mul
    into oT, all off the PE critical path.
  - output projection in the tail in plain 128x128 mode (ACT evicts).
"""

import os
import sys

sys.path.insert(0, "/opt/trn_rl_repo")

import numpy as np

import concourse.bass as bass
from concourse import bacc
import concourse.mybir as mybir
import concourse.tile as tile

F32 = mybir.dt.float32
BF16 = mybir.dt.float16   # compute dtype: fp16
BF = np.float16

H = 2048        # hidden
T = 1024        # tokens per block
NH = 32         # q heads
NKV = 8         # kv heads
D = 64          # head dim
KT = H // 128   # 16 hidden k-tiles
TT = T // 128   # 8 token tiles
NPAIR = NH // 2  # 16 head pairs
SCALE = D ** -0.5

LAST_EXEC_TIME_NS = None
LAST_RESULTS = None
_CACHED_NC = None


def build_nc(e_bufs=18, qp_bufs=2, wq_bufs=3):
    nc = bacc.Bacc("TRN2")
    xT = nc.dram_tensor("xT", [H, T], BF16, kind="ExternalInput")
    wq = nc.dram_tensor("wq", [H, H], BF16, kind="ExternalInput")
    wk = nc.dram_tensor("wk", [H, NKV * D], BF16, kind="ExternalInput")
    wv = nc.dram_tensor("wv", [H, NKV * D], BF16, kind="ExternalInput")
    wo = nc.dram_tensor("wo", [H, H], BF16, kind="ExternalInput")
    out = nc.dram_tensor("out", [T, H], F32, kind="ExternalOutput")
    # bounce scratch: [j, 0] = dens (2 heads x T), [j, 1] = reciprocals
    scr = nc.dram_tensor("scr", [NPAIR, 2, 2, T], F32)

    xT_v = xT.ap().rearrange("(k p) t -> p k t", p=128)
    wq_v = wq.ap().rearrange("(k p) m -> p k m", p=128)
    wk_v = wk.ap().rearrange("(k p) m -> p k m", p=128)
    wv_v = wv.ap().rearrange("(k p) m -> p k m", p=128)
    wo_v = wo.ap().rearrange("(k p) m -> p k m", p=128)

    from contextlib import ExitStack
    with tile.TileContext(nc) as tc:
        with ExitStack() as stack:
            oT_pool = stack.enter_context(tc.tile_pool(name="oT", bufs=1))
            xT_pool = stack.enter_context(tc.tile_pool(name="xTs", bufs=1))
            kdup_pool = stack.enter_context(tc.tile_pool(name="kdup", bufs=1))
            vplus_pool = stack.enter_context(tc.tile_pool(name="vplus", bufs=1))
            wvk_pool = stack.enter_context(tc.tile_pool(name="wvk", bufs=1))
            wk_pool = stack.enter_context(tc.tile_pool(name="wks", bufs=2))
            kT_pool = stack.enter_context(tc.tile_pool(name="kTs", bufs=2))
            pp = stack.enter_context(tc.tile_pool(name="pp", bufs=2, space="PSUM"))
            qq = stack.enter_context(tc.tile_pool(name="qq", bufs=2, space="PSUM"))
            oT = oT_pool.tile([128, KT, T], BF16)
            xTs = xT_pool.tile([128, KT, T], BF16)
            kdup = kdup_pool.tile([128, NKV, T], BF16)  # k_g^T on both halves
            vplus = vplus_pool.tile([128, TT, NKV, D + 1], BF16)
            wv_s = wvk_pool.tile([128, KT, 512], BF16)

            for k8 in range(8):
                nc.sync.dma_start(
                    out=xTs[:, 2 * k8:2 * (k8 + 1), :],
                    in_=xT_v[:, 2 * k8:2 * (k8 + 1), :],
                )
            for k4 in range(4):
                nc.sync.dma_start(
                    out=wv_s[:, 4 * k4:4 * (k4 + 1), :],
                    in_=wv_v[:, 4 * k4:4 * (k4 + 1), :],
                )
            nc.vector.memset(vplus[:, :, :, D:D + 1], 1.0)

            # ---------------- main pair loop ----------------
            with ExitStack() as mstack:
                wq_pool = mstack.enter_context(
                    tc.tile_pool(name="wqs", bufs=wq_bufs))
                qpair_pool = mstack.enter_context(
                    tc.tile_pool(name="qpair", bufs=qp_bufs))
                e_pool = mstack.enter_context(
                    tc.tile_pool(name="e", bufs=e_bufs))
                ou_pool = mstack.enter_context(
                    tc.tile_pool(name="ou", bufs=2))
                den_pool = mstack.enter_context(
                    tc.tile_pool(name="den", bufs=2))
                tmp_pool = mstack.enter_context(
                    tc.tile_pool(name="tmp", bufs=2))
                rec_pool = mstack.enter_context(
                    tc.tile_pool(name="rec", bufs=2))
                rbc_pool = mstack.enter_context(
                    tc.tile_pool(name="rbc", bufs=2))
                qk_pool = mstack.enter_context(
                    tc.tile_pool(name="qk", bufs=2, space="PSUM"))
                def paired_accum(ps_lo, ps_hi, lhs_of_k, rhs_of_k, n_k):
                    """(64,128)-mode dual-bank accumulation: contraction rows
                    0-63 -> ps_lo, rows 64-127 -> ps_hi; 2-MM union-stream
                    slots."""
                    for i in range(n_k):
                        st, sp = (i == 0), (i == n_k - 1)
                        lhs = lhs_of_k(i)
                        rhs = rhs_of_k(i)
                        nc.tensor.matmul(ps_lo, lhs[0:64], rhs[0:64],
                                         start=st, stop=sp)
                        nc.tensor.matmul(ps_hi, lhs[64:128], rhs[64:128],
                                         start=st, stop=sp)

                def kproj_block(m):
                    # kT_blk = (Wk[:, 128m:128(m+1)])^T @ x   [128, T]
                    wk_s = wk_pool.tile([128, KT, 128], BF16, tag="wk_s")
                    nc.sync.dma_start(
                        out=wk_s, in_=wk_v[:, :, 128 * m:128 * (m + 1)])
                    kT_blk = kT_pool.tile([128, T], BF16, tag="kT")
                    for n in range(2):
                        plo = pp.tile([128, 512], F32, tag="pp")
                        phi = qq.tile([128, 512], F32, tag="qq")
                        paired_accum(
                            plo, phi,
                            lambda k: wk_s[:, k, :],
                            lambda k: xTs[:, k, 512 * n:512 * (n + 1)],
                            KT)
                        tmp = tmp_pool.tile([128, 512], F32, tag="tmp")
                        nc.vector.tensor_copy(tmp, phi)
                        nc.vector.tensor_add(
                            kT_blk[:, 512 * n:512 * (n + 1)], plo, tmp)
                    for gg in range(2):
                        g = 2 * m + gg
                        src = kT_blk[64 * gg:64 * gg + 64, :]
                        nc.sync.dma_start(out=kdup[0:64, g, :], in_=src)
                        nc.sync.dma_start(out=kdup[64:128, g, :], in_=src)

                def vproj_mtile(m):
                    # vplus[:, m, :, 0:64] = (x^T)[128m:128(m+1), :] @ Wv
                    plo = pp.tile([128, 512], F32, tag="pp")
                    phi = qq.tile([128, 512], F32, tag="qq")
                    paired_accum(
                        plo, phi,
                        lambda k: xTs[:, k, 128 * m:128 * (m + 1)],
                        lambda k: wv_s[:, k, :],
                        KT)
                    tmp = tmp_pool.tile([128, 512], F32, tag="tmp")
                    nc.vector.tensor_copy(tmp, phi)
                    nc.vector.tensor_add(
                        vplus[:, m, :, 0:D],
                        plo.rearrange("p (h d) -> p h d", h=NKV),
                        tmp.rearrange("p (h d) -> p h d", h=NKV))

                for j in range(NPAIR):
                    g = j // 2
                    if j % 4 == 0:
                        kproj_block(j // 4)
                    if j == 0:
                        for m in range(TT):
                            vproj_mtile(m)

                    # q projection for this pair (heads 2j, 2j+1)
                    wq_s = wq_pool.tile([128, KT, 128], BF16, tag="wq_s")
                    nc.sync.dma_start(out=wq_s,
                                      in_=wq_v[:, :, 128 * j:128 * (j + 1)])
                    qpair = qpair_pool.tile([128, T], BF16, tag="qpair")
                    for n in range(2):
                        plo = pp.tile([128, 512], F32, tag="pp")
                        phi = qq.tile([128, 512], F32, tag="qq")
                        paired_accum(
                            plo, phi,
                            lambda k: wq_s[:, k, :],
                            lambda k: xTs[:, k, 512 * n:512 * (n + 1)],
                            KT)
                        tmp = tmp_pool.tile([128, 512], F32, tag="tmp")
                        nc.vector.tensor_copy(tmp, phi)
                        nc.vector.tensor_add(
                            qpair[:, 512 * n:512 * (n + 1)], plo, tmp)

                    # QK^T + exp: head A -> qkA, head B -> qkB; the A/B MMs
                    # share the union moving stream qpair[:, n-half] -> 2x.
                    e_tiles = [[None] * TT, [None] * TT]
                    for t in range(TT):
                        qkA = qk_pool.tile([128, T], F32, tag="qk")
                        qkB = qk_pool.tile([128, T], F32, tag="qk")
                        for n in range(2):
                            nc.tensor.matmul(
                                qkA[:, 512 * n:512 * (n + 1)],
                                kdup[0:64, g, 128 * t:128 * (t + 1)],
                                qpair[0:64, 512 * n:512 * (n + 1)],
                                start=True, stop=True,
                            )
                            nc.tensor.matmul(
                                qkB[:, 512 * n:512 * (n + 1)],
                                kdup[64:128, g, 128 * t:128 * (t + 1)],
                                qpair[64:128, 512 * n:512 * (n + 1)],
                                start=True, stop=True,
                            )
                        eA = e_pool.tile([128, T], BF16, tag="e")
                        nc.scalar.activation(
                            eA, qkA, mybir.ActivationFunctionType.Exp,
                            scale=SCALE)
                        eB = e_pool.tile([128, T], BF16, tag="e")
                        nc.scalar.activation(
                            eB, qkB, mybir.ActivationFunctionType.Exp,
                            scale=SCALE)
                        e_tiles[0][t] = eA
                        e_tiles[1][t] = eB

                    # PV: per (hh, n) accumulate over 8 token chunks; K=128
                    # split t-lo -> psX, t-hi -> psY (union stream e[:, n]).
                    ou = ou_pool.tile([128, T], F32, tag="ou")
                    dens = [None, None]
                    for hh in range(2):
                        den_t = den_pool.tile([65, T], F32, tag="den")
                        dens[hh] = den_t
                        for n in range(2):
                            psX = pp.tile([65, 512], F32, tag="pp")
                            psY = qq.tile([65, 512], F32, tag="qq")
                            for c in range(TT):
                                st, sp = (c == 0), (c == TT - 1)
                                e_t = e_tiles[hh][c]
                                nc.tensor.matmul(
                                    psX, vplus[0:64, c, g, :],
                                    e_t[0:64, 512 * n:512 * (n + 1)],
                                    start=st, stop=sp)
                                nc.tensor.matmul(
                                    psY, vplus[64:128, c, g, :],
                                    e_t[64:128, 512 * n:512 * (n + 1)],
                                    start=st, stop=sp)
                            tmp = tmp_pool.tile([65, 512], F32, tag="tmpv")
                            nc.vector.tensor_copy(tmp, psY)
                            nc.vector.tensor_add(
                                ou[64 * hh:64 * hh + 64,
                                   512 * n:512 * (n + 1)],
                                psX[0:64, :], tmp[0:64, :])
                            nc.vector.tensor_add(
                                den_t[64:65, 512 * n:512 * (n + 1)],
                                psX[64:65, :], tmp[64:65, :])
                        # den row (partition 64) -> DRAM bounce
                        nc.sync.dma_start(
                            out=scr.ap()[j, 0, hh], in_=den_t[64:65, :])

                    # reciprocal via spread-bounce: scr[j,0] (2048 f32) ->
                    # [128, 16] -> recip -> scr[j,1] -> broadcast rbc
                    den_sp = rec_pool.tile([128, 16], F32, tag="den_sp")
                    nc.sync.dma_start(
                        out=den_sp,
                        in_=scr.ap()[j, 0].rearrange("a b -> (a b)")
                        .rearrange("(p e) -> p e", p=128),
                    )
                    rec_sp = rec_pool.tile([128, 16], F32, tag="rec_sp")
                    nc.vector.reciprocal(rec_sp, den_sp)
                    nc.sync.dma_start(
                        out=scr.ap()[j, 1].rearrange("a b -> (a b)")
                        .rearrange("(p e) -> p e", p=128),
                        in_=rec_sp,
                    )
                    rbc = rbc_pool.tile([128, T], F32, tag="rbc")
                    for hh in range(2):
                        bsrc = bass.AP(
                            tensor=scr.ap().tensor,
                            offset=(j * 4 + 2 + hh) * T,
                            ap=[[0, 64], [1, T]],
                        )
                        nc.sync.dma_start(
                            out=rbc[64 * hh:64 * hh + 64, :], in_=bsrc)
                    for n in range(2):
                        nc.gpsimd.tensor_mul(
                            oT[:, j, 512 * n:512 * (n + 1)],
                            ou[:, 512 * n:512 * (n + 1)],
                            rbc[:, 512 * n:512 * (n + 1)],
                        )

            # ---------------- output projection (tail, 128x128 mode) -------
            with ExitStack() as tstack:
                wo_pool = tstack.enter_context(tc.tile_pool(name="wos", bufs=2))
                ob_pool = tstack.enter_context(tc.tile_pool(name="ob", bufs=4))
                ops = tstack.enter_context(
                    tc.tile_pool(name="ops", bufs=4, space="PSUM"))
                for c in range(4):
                    wo_s = wo_pool.tile([128, KT, 512], BF16, tag="wo_s")
                    nc.sync.dma_start(
                        out=wo_s, in_=wo_v[:, :, 512 * c:512 * (c + 1)]
                    )
                    for m in range(TT):
                        ps = ops.tile([128, 512], F32, tag="ops")
                        for k in range(KT):
                            nc.tensor.matmul(
                                ps, oT[:, k, 128 * m:128 * (m + 1)],
                                wo_s[:, k, :],
                                start=(k == 0), stop=(k == KT - 1),
                            )
                        ob = ob_pool.tile([128, 512], F32, tag="ob")
                        nc.scalar.copy(ob, ps)
                        nc.sync.dma_start(
                            out=out.ap()[128 * m:128 * (m + 1),
                                         512 * c:512 * (c + 1)],
                            in_=ob,
                        )
    nc.finalize()
    return nc


def _prep_inputs(hidden_states, Wq, Wk, Wv, Wo):
    hs = np.asarray(hidden_states, dtype=np.float32)
    B = hs.shape[0]
    # token index l = ix*2048 + sx*64 + iy*32 + sy  (bnx=2, BSX=32, bny=2, BSY=32)
    hsv = hs.reshape(B, 2, 32, 2, 32, H)  # b ix sx iy sy h
    wq_b = np.asarray(Wq, dtype=np.float32).astype(BF)
    wk_b = np.asarray(Wk, dtype=np.float32).astype(BF)
    wv_b = np.asarray(Wv, dtype=np.float32).astype(BF)
    wo_b = np.asarray(Wo, dtype=np.float32).astype(BF)
    in_maps = []
    for c in range(8):
        b, ix, iy = c // 4, (c // 2) % 2, c % 2
        x_blk = hsv[b, ix, :, iy, :, :].reshape(T, H)
        xT = np.ascontiguousarray(x_blk.T).astype(BF)
        in_maps.append({"xT": xT, "wq": wq_b, "wk": wk_b, "wv": wv_b, "wo": wo_b})
    return in_maps


def kernel(hidden_states, Wq, Wk, Wv, Wo, x_dim=64, y_dim=64):
    global LAST_EXEC_TIME_NS, LAST_RESULTS, _CACHED_NC
    assert int(x_dim) == 64 and int(y_dim) == 64

    from concourse.bass_utils import run_bass_kernel_spmd

    if _CACHED_NC is None:
        _CACHED_NC = build_nc()
    nc = _CACHED_NC

    in_maps = _prep_inputs(hidden_states, Wq, Wk, Wv, Wo)
    trace = bool(os.environ.get("BASS_TRACE"))
    res = run_bass_kernel_spmd(nc, in_maps, core_ids=list(range(8)), trace=trace)
    LAST_EXEC_TIME_NS = res.exec_time_ns
    LAST_RESULTS = res
    out = np.concatenate([r["out"] for r in res.results], axis=0)
    return np.ascontiguousarray(out.reshape(2, 4096, H).astype(np.float32))


# revision 10
# speedup vs baseline: 1.2488x; 1.2488x over previous
"""Block2D attention on 8 TRN2 NeuronCores (fp16 compute, fp32 accum).

Sharding: data-parallel over the 8 independent (b, bnx, bny) attention blocks
(B=2 x bnx=2 x bny=2), one block of T=1024 tokens per core. No collectives.

Per-core pipeline (fp16 matmuls, fp32 PSUM accumulation):
  - q/k/v projections and PV run in plain 128x128 mode, single psum bank,
    dedicated pools (pp for projections, pv for PV) -- identical to the
    fastest-known baseline structure.
  - QK^T is the one place with a real 2x: K=64 per head, so head A runs on
    PE rows 0-63 -> bank qkA while head B runs on rows 64-127 -> bank qkB.
    Their moving operands are complementary partition halves of qpair[:, n]
    at the same free offset, which the HW merges into ONE 512-col stream:
    both matmuls execute concurrently (measured pair = ~216+6 ns).
    To keep the pair co-ready (so the scheduler issues them back-to-back),
    the qkA/qkB buffer allocation order alternates per t: the first-issued
    member (A) always lands on the buffer freed by the LATER exp-ACT of the
    previous t, so B is never the one stalling.
  - exp on ACT as [128,1024] instructions (~1147 ns each, ~293us total --
    the main-loop pacing engine).
  - PV: K=128 token chunks, stationary vplus [tokens, 65] (64 v dims + ones
    column); psum [65,512]: rows 0-63 unnormalized o, row 64 the softmax
    denominator. DVE copy-evicts into pair-packed ou [128, 1024] and a
    partition-64-aligned den row; reciprocal via DRAM spread-bounce;
    GPSIMD normalize-mul into oT. All off the PE critical path.
  - k/v projections staggered inside the main loop (k per 2-group block
    right before first use; v during pair 0) to keep the lead-in short.
  - output projection in the tail, 128x128 mode, ACT copy-evicts.
"""

import os
import sys

sys.path.insert(0, "/opt/trn_rl_repo")

import numpy as np

import concourse.bass as bass
from concourse import bacc
import concourse.mybir as mybir
import concourse.tile as tile

F32 = mybir.dt.float32
BF16 = mybir.dt.float16   # compute dtype: fp16
BF = np.float16

H = 2048        # hidden
T = 1024        # tokens per block
NH = 32         # q heads
NKV = 8         # kv heads
D = 64          # head dim
KT = H // 128   # 16 hidden k-tiles
TT = T // 128   # 8 token tiles
NPAIR = NH // 2  # 16 head pairs
SCALE = D ** -0.5

LAST_EXEC_TIME_NS = None
LAST_RESULTS = None
_CACHED_NC = None


def build_nc(e_bufs=18, qp_bufs=3, wq_bufs=3):
    nc = bacc.Bacc("TRN2")
    xT = nc.dram_tensor("xT", [H, T], BF16, kind="ExternalInput")
    wq = nc.dram_tensor("wq", [H, H], BF16, kind="ExternalInput")
    wk = nc.dram_tensor("wk", [H, NKV * D], BF16, kind="ExternalInput")
    wv = nc.dram_tensor("wv", [H, NKV * D], BF16, kind="ExternalInput")
    wo = nc.dram_tensor("wo", [H, H], BF16, kind="ExternalInput")
    out = nc.dram_tensor("out", [T, H], F32, kind="ExternalOutput")
    # bounce scratch: [j, 0] = dens (2 heads x T), [j, 1] = reciprocals
    scr = nc.dram_tensor("scr", [NPAIR, 2, 2, T], F32)

    xT_v = xT.ap().rearrange("(k p) t -> p k t", p=128)
    wq_v = wq.ap().rearrange("(k p) m -> p k m", p=128)
    wk_v = wk.ap().rearrange("(k p) m -> p k m", p=128)
    wv_v = wv.ap().rearrange("(k p) m -> p k m", p=128)
    wo_v = wo.ap().rearrange("(k p) m -> p k m", p=128)

    from contextlib import ExitStack
    with tile.TileContext(nc) as tc:
        with ExitStack() as stack:
            oT_pool = stack.enter_context(tc.tile_pool(name="oT", bufs=1))
            xT_pool = stack.enter_context(tc.tile_pool(name="xTs", bufs=1))
            kdup_pool = stack.enter_context(tc.tile_pool(name="kdup", bufs=1))
            vplus_pool = stack.enter_context(tc.tile_pool(name="vplus", bufs=1))
            wvk_pool = stack.enter_context(tc.tile_pool(name="wvk", bufs=1))
            wk_pool = stack.enter_context(tc.tile_pool(name="wks", bufs=2))
            kT_pool = stack.enter_context(tc.tile_pool(name="kTs", bufs=2))
            pp = stack.enter_context(tc.tile_pool(name="pp", bufs=2, space="PSUM"))

            oT = oT_pool.tile([128, KT, T], BF16)
            xTs = xT_pool.tile([128, KT, T], BF16)
            kdup = kdup_pool.tile([128, NKV, T], BF16)  # k_g^T on both halves
            vplus = vplus_pool.tile([128, TT, NKV, D + 1], BF16)
            wv_s = wvk_pool.tile([128, KT, 512], BF16)

            for k8 in range(8):
                nc.sync.dma_start(
                    out=xTs[:, 2 * k8:2 * (k8 + 1), :],
                    in_=xT_v[:, 2 * k8:2 * (k8 + 1), :],
                )
            for k4 in range(4):
                nc.sync.dma_start(
                    out=wv_s[:, 4 * k4:4 * (k4 + 1), :],
                    in_=wv_v[:, 4 * k4:4 * (k4 + 1), :],
                )
            nc.vector.memset(vplus[:, :, :, D:D + 1], 1.0)

            # ---------------- main pair loop ----------------
            with ExitStack() as mstack:
                wq_pool = mstack.enter_context(
                    tc.tile_pool(name="wqs", bufs=wq_bufs))
                qpair_pool = mstack.enter_context(
                    tc.tile_pool(name="qpair", bufs=qp_bufs))
                e_pool = mstack.enter_context(
                    tc.tile_pool(name="e", bufs=e_bufs))
                ou_pool = mstack.enter_context(
                    tc.tile_pool(name="ou", bufs=2))
                den_pool = mstack.enter_context(
                    tc.tile_pool(name="den", bufs=2))
                rec_pool = mstack.enter_context(
                    tc.tile_pool(name="rec", bufs=2))
                rbc_pool = mstack.enter_context(
                    tc.tile_pool(name="rbc", bufs=2))
                qk_pool = mstack.enter_context(
                    tc.tile_pool(name="qk", bufs=2, space="PSUM"))
                pv_pool = mstack.enter_context(
                    tc.tile_pool(name="pv", bufs=2, space="PSUM"))

                def kproj_block(m):
                    # kT_blk = (Wk[:, 128m:128(m+1)])^T @ x   [128, T]
                    wk_s = wk_pool.tile([128, KT, 128], BF16, tag="wk_s")
                    nc.sync.dma_start(
                        out=wk_s, in_=wk_v[:, :, 128 * m:128 * (m + 1)])
                    kT_blk = kT_pool.tile([128, T], BF16, tag="kT")
                    for n in range(2):
                        ps = pp.tile([128, 512], F32, tag="pp")
                        for k in range(KT):
                            nc.tensor.matmul(
                                ps, wk_s[:, k, :],
                                xTs[:, k, 512 * n:512 * (n + 1)],
                                start=(k == 0), stop=(k == KT - 1))
                        nc.vector.tensor_copy(
                            kT_blk[:, 512 * n:512 * (n + 1)], ps)
                    for gg in range(2):
                        g = 2 * m + gg
                        src = kT_blk[64 * gg:64 * gg + 64, :]
                        nc.sync.dma_start(out=kdup[0:64, g, :], in_=src)
                        nc.sync.dma_start(out=kdup[64:128, g, :], in_=src)

                def vproj_mtile(m):
                    # vplus[:, m, :, 0:64] = (x^T)[128m:128(m+1), :] @ Wv
                    ps = pp.tile([128, 512], F32, tag="pp")
                    for k in range(KT):
                        nc.tensor.matmul(
                            ps, xTs[:, k, 128 * m:128 * (m + 1)],
                            wv_s[:, k, :],
                            start=(k == 0), stop=(k == KT - 1))
                    nc.vector.tensor_copy(
                        vplus[:, m, :, 0:D],
                        ps.rearrange("p (h d) -> p h d", h=NKV))

                for j in range(NPAIR):
                    g = j // 2
                    if j % 4 == 0:
                        kproj_block(j // 4)
                    if j == 0:
                        for m in range(TT):
                            vproj_mtile(m)

                    # q projection for this pair (heads 2j, 2j+1)
                    wq_s = wq_pool.tile([128, KT, 128], BF16, tag="wq_s")
                    nc.sync.dma_start(out=wq_s,
                                      in_=wq_v[:, :, 128 * j:128 * (j + 1)])
                    qpair = qpair_pool.tile([128, T], BF16, tag="qpair")
                    for n in range(2):
                        ps = pp.tile([128, 512], F32, tag="pp")
                        for k in range(KT):
                            nc.tensor.matmul(
                                ps, wq_s[:, k, :],
                                xTs[:, k, 512 * n:512 * (n + 1)],
                                start=(k == 0), stop=(k == KT - 1))
                        nc.vector.tensor_copy(
                            qpair[:, 512 * n:512 * (n + 1)], ps)

                    # QK^T + exp. Head A (rows 0-63) and head B (rows 64-127)
                    # share one union moving stream per n -> concurrent pair.
                    # Allocation order alternates per t so the first-issued
                    # member (A) waits on the LATER previous ACT.
                    e_tiles = [[None] * TT, [None] * TT]
                    for t in range(TT):
                        if t % 2 == 0:
                            qkA = qk_pool.tile([128, T], F32, tag="qk")
                            qkB = qk_pool.tile([128, T], F32, tag="qk")
                        else:
                            qkB = qk_pool.tile([128, T], F32, tag="qk")
                            qkA = qk_pool.tile([128, T], F32, tag="qk")
                        for n in range(2):
                            nc.tensor.matmul(
                                qkA[:, 512 * n:512 * (n + 1)],
                                kdup[0:64, g, 128 * t:128 * (t + 1)],
                                qpair[0:64, 512 * n:512 * (n + 1)],
                                start=True, stop=True,
                            )
                            nc.tensor.matmul(
                                qkB[:, 512 * n:512 * (n + 1)],
                                kdup[64:128, g, 128 * t:128 * (t + 1)],
                                qpair[64:128, 512 * n:512 * (n + 1)],
                                start=True, stop=True,
                            )
                        eA = e_pool.tile([128, T], BF16, tag="e")
                        nc.scalar.activation(
                            eA, qkA, mybir.ActivationFunctionType.Exp,
                            scale=SCALE)
                        eB = e_pool.tile([128, T], BF16, tag="e")
                        nc.scalar.activation(
                            eB, qkB, mybir.ActivationFunctionType.Exp,
                            scale=SCALE)
                        e_tiles[0][t] = eA
                        e_tiles[1][t] = eB

                    # PV: per (hh, n) accumulate over 8 token chunks (K=128);
                    # psum [65, 512]: rows 0-63 o_unnorm, row 64 denominator.
                    ou = ou_pool.tile([128, T], F32, tag="ou")
                    dens = [None, None]
                    for hh in range(2):
                        den_t = den_pool.tile([65, T], F32, tag="den")
                        dens[hh] = den_t
                        for n in range(2):
                            ps = pv_pool.tile([65, 512], F32, tag="pv")
                            for c in range(TT):
                                nc.tensor.matmul(
                                    ps, vplus[:, c, g, :],
                                    e_tiles[hh][c][:, 512 * n:512 * (n + 1)],
                                    start=(c == 0), stop=(c == TT - 1))
                            nc.vector.tensor_copy(
                                ou[64 * hh:64 * hh + 64,
                                   512 * n:512 * (n + 1)],
                                ps[0:64, :])
                            nc.vector.tensor_copy(
                                den_t[64:65, 512 * n:512 * (n + 1)],
                                ps[64:65, :])
                        nc.sync.dma_start(
                            out=scr.ap()[j, 0, hh], in_=den_t[64:65, :])

                    # reciprocal via spread-bounce: scr[j,0] (2048 f32) ->
                    # [128, 16] -> recip -> scr[j,1] -> broadcast rbc
                    den_sp = rec_pool.tile([128, 16], F32, tag="den_sp")
                    nc.sync.dma_start(
                        out=den_sp,
                        in_=scr.ap()[j, 0].rearrange("a b -> (a b)")
                        .rearrange("(p e) -> p e", p=128),
                    )
                    rec_sp = rec_pool.tile([128, 16], F32, tag="rec_sp")
                    nc.vector.reciprocal(rec_sp, den_sp)
                    nc.sync.dma_start(
                        out=scr.ap()[j, 1].rearrange("a b -> (a b)")
                        .rearrange("(p e) -> p e", p=128),
                        in_=rec_sp,
                    )
                    rbc = rbc_pool.tile([128, T], F32, tag="rbc")
                    for hh in range(2):
                        bsrc = bass.AP(
                            tensor=scr.ap().tensor,
                            offset=(j * 4 + 2 + hh) * T,
                            ap=[[0, 64], [1, T]],
                        )
                        nc.sync.dma_start(
                            out=rbc[64 * hh:64 * hh + 64, :], in_=bsrc)
                    for n in range(2):
                        nc.gpsimd.tensor_mul(
                            oT[:, j, 512 * n:512 * (n + 1)],
                            ou[:, 512 * n:512 * (n + 1)],
                            rbc[:, 512 * n:512 * (n + 1)],
                        )

            # ---------------- output projection (tail, 128x128 mode) -------
            with ExitStack() as tstack:
                wo_pool = tstack.enter_context(tc.tile_pool(name="wos", bufs=2))
                ob_pool = tstack.enter_context(tc.tile_pool(name="ob", bufs=4))
                ops = tstack.enter_context(
                    tc.tile_pool(name="ops", bufs=4, space="PSUM"))
                for c in range(4):
                    wo_s = wo_pool.tile([128, KT, 512], BF16, tag="wo_s")
                    nc.sync.dma_start(
                        out=wo_s, in_=wo_v[:, :, 512 * c:512 * (c + 1)]
                    )
                    for m in range(TT):
                        ps = ops.tile([128, 512], F32, tag="ops")
                        for k in range(KT):
                            nc.tensor.matmul(
                                ps, oT[:, k, 128 * m:128 * (m + 1)],
                                wo_s[:, k, :],
                                start=(k == 0), stop=(k == KT - 1),
                            )
                        ob = ob_pool.tile([128, 512], F32, tag="ob")
                        nc.scalar.copy(ob, ps)
                        nc.sync.dma_start(
                            out=out.ap()[128 * m:128 * (m + 1),
                                         512 * c:512 * (c + 1)],
                            in_=ob,
                        )
    nc.finalize()
    return nc


def _prep_inputs(hidden_states, Wq, Wk, Wv, Wo):
    hs = np.asarray(hidden_states, dtype=np.float32)
    B = hs.shape[0]
    # token index l = ix*2048 + sx*64 + iy*32 + sy  (bnx=2, BSX=32, bny=2, BSY=32)
    hsv = hs.reshape(B, 2, 32, 2, 32, H)  # b ix sx iy sy h
    wq_b = np.asarray(Wq, dtype=np.float32).astype(BF)
    wk_b = np.asarray(Wk, dtype=np.float32).astype(BF)
    wv_b = np.asarray(Wv, dtype=np.float32).astype(BF)
    wo_b = np.asarray(Wo, dtype=np.float32).astype(BF)
    in_maps = []
    for c in range(8):
        b, ix, iy = c // 4, (c // 2) % 2, c % 2
        x_blk = hsv[b, ix, :, iy, :, :].reshape(T, H)
        xT = np.ascontiguousarray(x_blk.T).astype(BF)
        in_maps.append({"xT": xT, "wq": wq_b, "wk": wk_b, "wv": wv_b, "wo": wo_b})
    return in_maps


def kernel(hidden_states, Wq, Wk, Wv, Wo, x_dim=64, y_dim=64):
    global LAST_EXEC_TIME_NS, LAST_RESULTS, _CACHED_NC
    assert int(x_dim) == 64 and int(y_dim) == 64

    from concourse.bass_utils import run_bass_kernel_spmd

    if _CACHED_NC is None:
        _CACHED_NC = build_nc()
    nc = _CACHED_NC

    in_maps = _prep_inputs(hidden_states, Wq, Wk, Wv, Wo)
    trace = bool(os.environ.get("BASS_TRACE"))
    res = run_bass_kernel_spmd(nc, in_maps, core_ids=list(range(8)), trace=trace)
    LAST_EXEC_TIME_NS = res.exec_time_ns
    LAST_RESULTS = res
    out = np.concatenate([r["out"] for r in res.results], axis=0)
    return np.ascontiguousarray(out.reshape(2, 4096, H).astype(np.float32))
